# revision 26
# baseline (speedup 1.0000x reference)
"""Trainium2 Bass kernel for nn_Compositional: sigmoid(sum(er*ea*eb, -1)).

  ea = x @ W_ent.T   [N, D]
  eb = y @ W_ent.T   [N, D]
  er = r @ W_rel.T   [N, D]
  out = sigmoid(sum_d er*ea*eb)  [N, 1]

Sharding: data-parallel over N across 8 cores (512 rows each), W_ent/W_rel
replicated.

Staging: all inputs are cast to bf16 and pre-transposed on the host so the
device streams [contraction, free] tiles directly (no PE transposes) at half
the HBM traffic of fp32.  Per-core DMA = xT 16MB + yT 16MB + wT 8MB + rT/wrT
0.75MB ~= 40.75MB (~113us at 360B/ns); PE = 2 GEMMs [512,16384]x[16384,256]
= 512 matmul instrs x 512 rows (~109us).  Memory/compute-balanced.

Per-core plan:
  - Everything computed transposed: eaT/ebT [D, n] with D on partitions.
  - Stream E in groups of 8 128-row chunks (w 0.5MB + x 1MB + y 1MB per
    group, double-buffered); per chunk, 4 accumulating matmuls
    (ea/eb x 2 d-halves) into persistent PSUM banks.
  - er phase after group 0 (small), stored to SBUF f32.
  - Epilogue: prod = eaT*ebT*erT on DVE, partition-reduce via ones-matmul,
    sigmoid on ACT, DMA out.
"""
import os

import numpy as np
import ml_dtypes

# Full-problem constants (hardcoded; kernel.py must be self-contained).
N, E, R, D = 4096, 16384, 512, 256
NCORES = 8
NC_N = N // NCORES      # 512 rows per core
GC = 8                  # 128-row e-chunks per DMA group
NCHUNK = E // 128       # 128 contraction chunks
NG = NCHUNK // GC       # 16 groups
DH = D // 128           # 2 d-halves
RC = R // 128           # 4 r-chunks

BF16NP = ml_dtypes.bfloat16

_CACHE = {}


def _build():
    import concourse.mybir as mybir
    import concourse.tile as tile
    from concourse import bacc

    F32 = mybir.dt.float32
    BF = mybir.dt.bfloat16
    MUL = mybir.AluOpType.mult

    nc = bacc.Bacc("TRN2", target_bir_lowering=False)

    xT_dram = nc.dram_tensor("xT", [E, NC_N], BF, kind="ExternalInput")
    yT_dram = nc.dram_tensor("yT", [E, NC_N], BF, kind="ExternalInput")
    rT_dram = nc.dram_tensor("rT", [R, NC_N], BF, kind="ExternalInput")
    wT_dram = nc.dram_tensor("wT", [E, D], BF, kind="ExternalInput")
    wrT_dram = nc.dram_tensor("wrT", [R, D], BF, kind="ExternalInput")
    out_dram = nc.dram_tensor("out", [NC_N, 1], F32, kind="ExternalOutput")

    with tile.TileContext(nc) as tc:
        with (
            tc.tile_pool(name="const", bufs=1) as cpool,
            tc.tile_pool(name="stream", bufs=1) as pool,
            tc.tile_pool(name="psum", bufs=1, space="PSUM") as psum,
        ):
            # ---- constants ----
            ones_bf = cpool.tile([128, 1], BF)
            nc.gpsimd.memset(ones_bf[:], 1.0)

            # dummy sigmoid up front: forces the sigmoid_and_others ACT table
            # (which also covers Copy) to load at t~0 instead of right before
            # the final sigmoid on the critical tail
            warm_sb = cpool.tile([1, 1], F32)
            nc.gpsimd.memset(warm_sb[:], 0.0)
            warm2_sb = cpool.tile([1, 1], F32)
            nc.scalar.activation(
                warm2_sb[:], warm_sb[:], mybir.ActivationFunctionType.Sigmoid
            )

            # erT for the epilogue products (f32, multiplied against PSUM f32)
            ert_sb = [cpool.tile([128, NC_N], F32, name=f"ert{dh}") for dh in range(DH)]

            # ---- PSUM accumulators (persist through main loop) ----
            ea_ps = [
                psum.tile([128, NC_N], F32, tag=f"ea{dh}", bufs=1, name=f"ea{dh}")
                for dh in range(DH)
            ]
            eb_ps = [
                psum.tile([128, NC_N], F32, tag=f"eb{dh}", bufs=1, name=f"eb{dh}")
                for dh in range(DH)
            ]

            def load_group(g, pieces=None):
                """DMA one e-group (GC chunks) of w/x/y, optionally split into
                `pieces` (chunk counts) so dependent matmuls can start early."""
                w_nat = pool.tile([128, GC, D], BF, tag="w_nat", bufs=3, name="w_nat")
                x_nat = pool.tile([128, GC, NC_N], BF, tag="x_nat", bufs=3, name="x_nat")
                y_nat = pool.tile([128, GC, NC_N], BF, tag="y_nat", bufs=3, name="y_nat")
                pieces = pieces or [GC]
                assert sum(pieces) == GC
                c0 = 0
                for pc in pieces:
                    cs = slice(c0, c0 + pc)
                    rs = slice((g * GC + c0) * 128, (g * GC + c0 + pc) * 128)
                    nc.sync.dma_start(
                        w_nat[:, cs, :],
                        wT_dram[rs, :].rearrange("(c p) d -> p c d", p=128),
                    )
                    nc.sync.dma_start(
                        x_nat[:, cs, :],
                        xT_dram[rs, :].rearrange("(c p) n -> p c n", p=128),
                    )
                    nc.sync.dma_start(
                        y_nat[:, cs, :],
                        yT_dram[rs, :].rearrange("(c p) n -> p c n", p=128),
                    )
                    c0 += pc
                return w_nat, x_nat, y_nat

            def mm_group(g, tiles, pieces=None, skip_last=0):
                """Per piece: all ea matmuls first (need only w+x), then eb
                (needs y) — so PE starts before the piece's y DMA lands.
                skip_last: leave the final `skip_last` chunks to the caller."""
                w_nat, x_nat, y_nat = tiles
                pieces = pieces or [GC]
                c0 = 0
                for pc in pieces:
                    hi = min(c0 + pc, GC - skip_last)
                    for c in range(c0, hi):
                        chunk = g * GC + c
                        for dh in range(DH):
                            nc.tensor.matmul(
                                ea_ps[dh][:],
                                w_nat[:, c, dh * 128 : (dh + 1) * 128],
                                x_nat[:, c, :],
                                start=(chunk == 0),
                                stop=False,
                            )
                    for c in range(c0, hi):
                        chunk = g * GC + c
                        for dh in range(DH):
                            nc.tensor.matmul(
                                eb_ps[dh][:],
                                w_nat[:, c, dh * 128 : (dh + 1) * 128],
                                y_nat[:, c, :],
                                start=(chunk == 0),
                                stop=False,
                            )
                    c0 += pc

            def rel_dma():
                wr_nat = pool.tile([128, RC, D], BF, tag="wr_nat", bufs=1, name="wr_nat")
                nc.sync.dma_start(
                    wr_nat[:], wrT_dram[:, :].rearrange("(c p) d -> p c d", p=128)
                )
                r_nat = pool.tile([128, RC, NC_N], BF, tag="r_nat", bufs=1, name="r_nat")
                nc.sync.dma_start(
                    r_nat[:], rT_dram[:, :].rearrange("(c p) n -> p c n", p=128)
                )
                return wr_nat, r_nat

            def rel_mm(tiles):
                wr_nat, r_nat = tiles
                er_ps = [
                    psum.tile([128, NC_N], F32, tag=f"er{dh}", bufs=1, name=f"er{dh}")
                    for dh in range(DH)
                ]
                for c in range(RC):
                    for dh in range(DH):
                        nc.tensor.matmul(
                            er_ps[dh][:],
                            wr_nat[:, c, dh * 128 : (dh + 1) * 128],
                            r_nat[:, c, :],
                            start=(c == 0),
                            stop=(c == RC - 1),
                        )
                nc.scalar.copy(ert_sb[0][:], er_ps[0][:])
                nc.scalar.copy(ert_sb[1][:], er_ps[1][:])

            # ---- main schedule ----
            # rel DMAs first: they fill the issue-pipeline warmup bubbles and
            # let the er matmuls act as PE ramp-up work
            rel = rel_dma()
            pg0 = [4, 4]
            g0 = load_group(0, pieces=pg0)
            rel_mm(rel)
            mm_group(0, g0, pieces=pg0)
            last_tiles = None
            for g in range(1, NG):
                # every group in pieces (halves PE's steady-state lag);
                # finer at the tail
                if g >= NG - 2:
                    pieces = [2, 2, 2, 2]
                else:
                    pieces = [4, 4]
                tiles = load_group(g, pieces=pieces)
                mm_group(g, tiles, pieces=pieces, skip_last=2 if g == NG - 1 else 0)
                last_tiles = tiles

            # ---- final 2 chunks + epilogue ----
            # ea finishes first (x-gated, staggered so ea0 stops earliest),
            # then t = ea*er runs on DVE while PE does the last eb matmuls,
            # then p = t*eb -> ones-matmul reduce, pipelined per d-half.
            w_nat, x_nat, y_nat = last_tiles
            c6, c7 = GC - 2, GC - 1
            for dh in range(DH):
                nc.tensor.matmul(
                    ea_ps[dh][:],
                    w_nat[:, c6, dh * 128 : (dh + 1) * 128],
                    x_nat[:, c6, :],
                    start=False,
                    stop=False,
                )
                nc.tensor.matmul(
                    ea_ps[dh][:],
                    w_nat[:, c7, dh * 128 : (dh + 1) * 128],
                    x_nat[:, c7, :],
                    start=False,
                    stop=True,
                )
            t_sb = []
            for dh in range(DH):
                t = pool.tile([128, NC_N], F32, tag="t_sb", bufs=2, name=f"t{dh}_sb")
                nc.vector.tensor_tensor(t[:], ea_ps[dh][:], ert_sb[dh][:], MUL)
                t_sb.append(t)
            for c in (c6, c7):
                for dh in range(DH):
                    nc.tensor.matmul(
                        eb_ps[dh][:],
                        w_nat[:, c, dh * 128 : (dh + 1) * 128],
                        y_nat[:, c, :],
                        start=False,
                        stop=(c == c7),
                    )
            score_ps = psum.tile([1, NC_N], F32, tag="score", bufs=1, name="score")
            for dh in range(DH):
                p_sb = pool.tile([128, NC_N], BF, tag="p_sb", bufs=2, name=f"p{dh}_sb")
                nc.vector.tensor_tensor(p_sb[:], eb_ps[dh][:], t_sb[dh][:], MUL)
                nc.tensor.matmul(
                    score_ps[:],
                    ones_bf[:],
                    p_sb[:],
                    start=(dh == 0),
                    stop=(dh == DH - 1),
                )
            sig_sb = pool.tile([1, NC_N], F32, name="sig_sb")
            nc.scalar.activation(
                sig_sb[:], score_ps[:], mybir.ActivationFunctionType.Sigmoid
            )
            nc.scalar.dma_start(out_dram[:].rearrange("n o -> o n"), sig_sb[:])

    nc.compile()
    return nc


def _get_nc():
    if "nc" not in _CACHE:
        _CACHE["nc"] = _build()
    return _CACHE["nc"]


def kernel(x, y, r, W_ent, W_rel):
    from concourse.bass_utils import run_bass_kernel_spmd

    x = np.asarray(x, dtype=np.float32)
    y = np.asarray(y, dtype=np.float32)
    r = np.asarray(r, dtype=np.float32)
    W_ent = np.asarray(W_ent, dtype=np.float32)
    W_rel = np.asarray(W_rel, dtype=np.float32)

    nc = _get_nc()

    wT_h = W_ent.T.astype(BF16NP, order="C")       # [E, D]
    wrT_h = W_rel.T.astype(BF16NP, order="C")      # [R, D]
    in_maps = []
    for c in range(NCORES):
        sl = slice(c * NC_N, (c + 1) * NC_N)
        in_maps.append(
            {
                "xT": x[sl].T.astype(BF16NP, order="C"),
                "yT": y[sl].T.astype(BF16NP, order="C"),
                "rT": r[sl].T.astype(BF16NP, order="C"),
                "wT": wT_h,
                "wrT": wrT_h,
            }
        )
    trace = bool(int(os.environ.get("KERNEL_TRACE", "0")))
    res = run_bass_kernel_spmd(
        nc, in_maps, core_ids=list(range(NCORES)), trace=trace
    )
    _CACHE["last_result"] = res
    out = np.concatenate([res.results[c]["out"] for c in range(NCORES)], axis=0)
    return out


# revision 27
# speedup vs baseline: 1.0011x; 1.0011x over previous
"""Trainium2 Bass kernel for nn_Compositional: sigmoid(sum(er*ea*eb, -1)).

  ea = x @ W_ent.T   [N, D]
  eb = y @ W_ent.T   [N, D]
  er = r @ W_rel.T   [N, D]
  out = sigmoid(sum_d er*ea*eb)  [N, 1]

Sharding: data-parallel over N across 8 cores (512 rows each), W_ent/W_rel
replicated.

Staging: all inputs are cast to bf16 and pre-transposed on the host so the
device streams [contraction, free] tiles directly (no PE transposes) at half
the HBM traffic of fp32.  Per-core DMA = xT 16MB + yT 16MB + wT 8MB + rT/wrT
0.75MB ~= 40.75MB (~113us at 360B/ns); PE = 2 GEMMs [512,16384]x[16384,256]
= 512 matmul instrs x 512 rows (~109us).  Memory/compute-balanced.

Per-core plan:
  - Everything computed transposed: eaT/ebT [D, n] with D on partitions.
  - Stream E in groups of 8 128-row chunks (w 0.5MB + x 1MB + y 1MB per
    group, double-buffered); per chunk, 4 accumulating matmuls
    (ea/eb x 2 d-halves) into persistent PSUM banks.
  - er phase after group 0 (small), stored to SBUF f32.
  - Epilogue: prod = eaT*ebT*erT on DVE, partition-reduce via ones-matmul,
    sigmoid on ACT, DMA out.
"""
import os

import numpy as np
import ml_dtypes

# Full-problem constants (hardcoded; kernel.py must be self-contained).
N, E, R, D = 4096, 16384, 512, 256
NCORES = 8
NC_N = N // NCORES      # 512 rows per core
GC = 8                  # 128-row e-chunks per DMA group
NCHUNK = E // 128       # 128 contraction chunks
NG = NCHUNK // GC       # 16 groups
DH = D // 128           # 2 d-halves
RC = R // 128           # 4 r-chunks

BF16NP = ml_dtypes.bfloat16

_CACHE = {}


def _build():
    import concourse.mybir as mybir
    import concourse.tile as tile
    from concourse import bacc

    F32 = mybir.dt.float32
    BF = mybir.dt.bfloat16
    MUL = mybir.AluOpType.mult

    nc = bacc.Bacc("TRN2", target_bir_lowering=False)

    xT_dram = nc.dram_tensor("xT", [E, NC_N], BF, kind="ExternalInput")
    yT_dram = nc.dram_tensor("yT", [E, NC_N], BF, kind="ExternalInput")
    rT_dram = nc.dram_tensor("rT", [R, NC_N], BF, kind="ExternalInput")
    wT_dram = nc.dram_tensor("wT", [E, D], BF, kind="ExternalInput")
    wrT_dram = nc.dram_tensor("wrT", [R, D], BF, kind="ExternalInput")
    out_dram = nc.dram_tensor("out", [NC_N, 1], F32, kind="ExternalOutput")

    with tile.TileContext(nc) as tc:
        with (
            tc.tile_pool(name="const", bufs=1) as cpool,
            tc.tile_pool(name="stream", bufs=1) as pool,
            tc.tile_pool(name="psum", bufs=1, space="PSUM") as psum,
        ):
            # ---- constants ----
            ones_bf = cpool.tile([128, 1], BF)
            nc.gpsimd.memset(ones_bf[:], 1.0)

            # dummy sigmoid up front: forces the sigmoid_and_others ACT table
            # (which also covers Copy) to load at t~0 instead of right before
            # the final sigmoid on the critical tail
            warm_sb = cpool.tile([1, 1], F32)
            nc.gpsimd.memset(warm_sb[:], 0.0)
            warm2_sb = cpool.tile([1, 1], F32)
            nc.scalar.activation(
                warm2_sb[:], warm_sb[:], mybir.ActivationFunctionType.Sigmoid
            )

            # erT for the epilogue products (f32, multiplied against PSUM f32)
            ert_sb = [cpool.tile([128, NC_N], F32, name=f"ert{dh}") for dh in range(DH)]

            # ---- PSUM accumulators (persist through main loop) ----
            ea_ps = [
                psum.tile([128, NC_N], F32, tag=f"ea{dh}", bufs=1, name=f"ea{dh}")
                for dh in range(DH)
            ]
            eb_ps = [
                psum.tile([128, NC_N], F32, tag=f"eb{dh}", bufs=1, name=f"eb{dh}")
                for dh in range(DH)
            ]

            def load_group(g, pieces=None):
                """DMA one e-group (GC chunks) of w/x/y, optionally split into
                `pieces` (chunk counts) so dependent matmuls can start early."""
                w_nat = pool.tile([128, GC, D], BF, tag="w_nat", bufs=3, name="w_nat")
                x_nat = pool.tile([128, GC, NC_N], BF, tag="x_nat", bufs=3, name="x_nat")
                y_nat = pool.tile([128, GC, NC_N], BF, tag="y_nat", bufs=3, name="y_nat")
                pieces = pieces or [GC]
                assert sum(pieces) == GC
                c0 = 0
                for pc in pieces:
                    cs = slice(c0, c0 + pc)
                    rs = slice((g * GC + c0) * 128, (g * GC + c0 + pc) * 128)
                    nc.sync.dma_start(
                        w_nat[:, cs, :],
                        wT_dram[rs, :].rearrange("(c p) d -> p c d", p=128),
                    )
                    nc.sync.dma_start(
                        x_nat[:, cs, :],
                        xT_dram[rs, :].rearrange("(c p) n -> p c n", p=128),
                    )
                    nc.sync.dma_start(
                        y_nat[:, cs, :],
                        yT_dram[rs, :].rearrange("(c p) n -> p c n", p=128),
                    )
                    c0 += pc
                return w_nat, x_nat, y_nat

            def mm_group(g, tiles, pieces=None, skip_last=0):
                """Per piece: all ea matmuls first (need only w+x), then eb
                (needs y) — so PE starts before the piece's y DMA lands.
                skip_last: leave the final `skip_last` chunks to the caller."""
                w_nat, x_nat, y_nat = tiles
                pieces = pieces or [GC]
                c0 = 0
                for pc in pieces:
                    hi = min(c0 + pc, GC - skip_last)
                    for c in range(c0, hi):
                        chunk = g * GC + c
                        for dh in range(DH):
                            nc.tensor.matmul(
                                ea_ps[dh][:],
                                w_nat[:, c, dh * 128 : (dh + 1) * 128],
                                x_nat[:, c, :],
                                start=(chunk == 0),
                                stop=False,
                            )
                    for c in range(c0, hi):
                        chunk = g * GC + c
                        for dh in range(DH):
                            nc.tensor.matmul(
                                eb_ps[dh][:],
                                w_nat[:, c, dh * 128 : (dh + 1) * 128],
                                y_nat[:, c, :],
                                start=(chunk == 0),
                                stop=False,
                            )
                    c0 += pc

            def rel_dma():
                wr_nat = pool.tile([128, RC, D], BF, tag="wr_nat", bufs=1, name="wr_nat")
                nc.sync.dma_start(
                    wr_nat[:], wrT_dram[:, :].rearrange("(c p) d -> p c d", p=128)
                )
                r_nat = pool.tile([128, RC, NC_N], BF, tag="r_nat", bufs=1, name="r_nat")
                nc.sync.dma_start(
                    r_nat[:], rT_dram[:, :].rearrange("(c p) n -> p c n", p=128)
                )
                return wr_nat, r_nat

            def rel_mm(tiles):
                wr_nat, r_nat = tiles
                er_ps = [
                    psum.tile([128, NC_N], F32, tag=f"er{dh}", bufs=1, name=f"er{dh}")
                    for dh in range(DH)
                ]
                for c in range(RC):
                    for dh in range(DH):
                        nc.tensor.matmul(
                            er_ps[dh][:],
                            wr_nat[:, c, dh * 128 : (dh + 1) * 128],
                            r_nat[:, c, :],
                            start=(c == 0),
                            stop=(c == RC - 1),
                        )
                nc.scalar.copy(ert_sb[0][:], er_ps[0][:])
                nc.scalar.copy(ert_sb[1][:], er_ps[1][:])

            # ---- main schedule ----
            # rel DMAs first: they fill the issue-pipeline warmup bubbles and
            # let the er matmuls act as PE ramp-up work
            rel = rel_dma()
            pg0 = [4, 4]
            g0 = load_group(0, pieces=pg0)
            rel_mm(rel)
            mm_group(0, g0, pieces=pg0)
            last_tiles = None
            for g in range(1, NG):
                # every group in pieces (halves PE's steady-state lag);
                # finer at the tail
                if g >= NG - 2:
                    pieces = [2, 2, 2, 2]
                else:
                    pieces = [4, 4]
                tiles = load_group(g, pieces=pieces)
                mm_group(g, tiles, pieces=pieces, skip_last=2 if g == NG - 1 else 0)
                last_tiles = tiles

            # ---- final 2 chunks + epilogue ----
            # ea finishes first (x-gated, staggered so ea0 stops earliest),
            # then t = ea*er runs on DVE while PE does the last eb matmuls,
            # then p = t*eb -> ones-matmul reduce, pipelined per d-half.
            w_nat, x_nat, y_nat = last_tiles
            c6, c7 = GC - 2, GC - 1
            for dh in range(DH):
                nc.tensor.matmul(
                    ea_ps[dh][:],
                    w_nat[:, c6, dh * 128 : (dh + 1) * 128],
                    x_nat[:, c6, :],
                    start=False,
                    stop=False,
                )
                nc.tensor.matmul(
                    ea_ps[dh][:],
                    w_nat[:, c7, dh * 128 : (dh + 1) * 128],
                    x_nat[:, c7, :],
                    start=False,
                    stop=True,
                )
            t_sb = []
            for dh in range(DH):
                t = pool.tile([128, NC_N], F32, tag="t_sb", bufs=2, name=f"t{dh}_sb")
                nc.vector.tensor_tensor(t[:], ea_ps[dh][:], ert_sb[dh][:], MUL)
                t_sb.append(t)
            for c in (c6, c7):
                for dh in range(DH):
                    nc.tensor.matmul(
                        eb_ps[dh][:],
                        w_nat[:, c, dh * 128 : (dh + 1) * 128],
                        y_nat[:, c, :],
                        start=False,
                        stop=(c == c7),
                    )
            score_ps = psum.tile([1, NC_N], F32, tag="score", bufs=1, name="score")
            for dh in range(DH):
                p_sb = pool.tile([128, NC_N], BF, tag="p_sb", bufs=2, name=f"p{dh}_sb")
                nc.vector.tensor_tensor(p_sb[:], eb_ps[dh][:], t_sb[dh][:], MUL)
                nc.tensor.matmul(
                    score_ps[:],
                    ones_bf[:],
                    p_sb[:],
                    start=(dh == 0),
                    stop=(dh == DH - 1),
                )
            sig_sb = pool.tile([1, NC_N], F32, name="sig_sb")
            nc.scalar.activation(
                sig_sb[:], score_ps[:], mybir.ActivationFunctionType.Sigmoid
            )
            nc.sync.dma_start(out_dram[:].rearrange("n o -> o n"), sig_sb[:])

    nc.compile()
    return nc


def _get_nc():
    if "nc" not in _CACHE:
        _CACHE["nc"] = _build()
    return _CACHE["nc"]


def kernel(x, y, r, W_ent, W_rel):
    from concourse.bass_utils import run_bass_kernel_spmd

    x = np.asarray(x, dtype=np.float32)
    y = np.asarray(y, dtype=np.float32)
    r = np.asarray(r, dtype=np.float32)
    W_ent = np.asarray(W_ent, dtype=np.float32)
    W_rel = np.asarray(W_rel, dtype=np.float32)

    nc = _get_nc()

    wT_h = W_ent.T.astype(BF16NP, order="C")       # [E, D]
    wrT_h = W_rel.T.astype(BF16NP, order="C")      # [R, D]
    in_maps = []
    for c in range(NCORES):
        sl = slice(c * NC_N, (c + 1) * NC_N)
        in_maps.append(
            {
                "xT": x[sl].T.astype(BF16NP, order="C"),
                "yT": y[sl].T.astype(BF16NP, order="C"),
                "rT": r[sl].T.astype(BF16NP, order="C"),
                "wT": wT_h,
                "wrT": wrT_h,
            }
        )
    trace = bool(int(os.environ.get("KERNEL_TRACE", "0")))
    res = run_bass_kernel_spmd(
        nc, in_maps, core_ids=list(range(NCORES)), trace=trace
    )
    _CACHE["last_result"] = res
    out = np.concatenate([res.results[c]["out"] for c in range(NCORES)], axis=0)
    return out
